# revision 25
# baseline (speedup 1.0000x reference)
"""MoE-routed low-rank attention (nn_NeuronCircuit_28930899706061) on 8 TRN2 cores.

Sharding: core c in 0..7 -> batch b = c//4, group position g = c%4. Phase 1
(routing + compress) processes 512 tokens per core, token-tile-interleaved:
local s-tile t in 0..3 holds global token tile 4t+g. This makes each half of a
core's local tiles a global token PREFIX (local tiles 0,1 across the 4-core
group = global tiles 0..7), so the h^T all-gather can be split into two
chunked collectives that overlap with compute: chunk A (global k/q tiles 0..7)
gathers while phase 1 finishes tiles 2,3; chunk B gathers while chunk-A
expand + attention (q tiles 0..7) runs. Attention is sharded by heads: core c
owns heads [4g, 4g+4) of batch b over all 2048 tokens.

Attention computes transposed score tiles P^T[k,q] directly on PE, so the
softmax exp itself performs the PSUM->SBUF move. Softmax denominators
accumulate via ones-column matmuls in their own PSUM bank; the PE stream is
software-pipelined (next group's scores are issued before the previous group's
z/AV). Mask multiplies run on DVE (the Pool engine runs the collectives and
must not block attention). Each core emits its four heads' unnormalized
attention outputs plus denominators; the host applies 1/z and the wO
projection and sums the 4 partials per batch.
"""

import numpy as np

B, S, D, H, RANK, NCMP = 2, 2048, 1024, 16, 128, 16
DH = D // H  # 64
N_CORES = 8
SHARD = S // 4  # 512 tokens per core in phase 1
HPC = 4  # heads per core
QT_TILES = S // 128  # 16 q tiles
GW = 4  # k-tiles per exp group (per-pair 1024-col exp = 2 PSUM banks)
NT4 = SHARD // 128  # 4 local s-tiles per core in phase 1
NCHUNK = 2  # collective chunks (each = NT4 // NCHUNK local tiles)
TPC = NT4 // NCHUNK  # local tiles per chunk
KPC = QT_TILES // NCHUNK  # global k/q tiles per chunk

_RUNNERS: dict = {}


def _split_multi_waits(nc, mybir):
    """This toolchain's walrus rejects any instruction carrying >1 sync wait
    ("Too many sync wait commands"); hoist excess waits onto same-engine nops
    inserted immediately before the instruction."""
    cnt = 0
    for f in nc.m.functions:
        for blk in f.blocks:
            il = blk.instructions
            out = []
            changed = False
            for inst in il:
                si = inst.sync_info
                waits = list(si.on_wait or []) if si else []
                if len(waits) > 1:
                    for w in waits[:-1]:
                        cnt += 1
                        nop = mybir.InstNoOp(
                            name=f"wsplit-{cnt}",
                            engine=inst.engine,
                            sync_info=mybir.SyncInfo(on_wait=[w], on_update=[]),
                        )
                        nc.register_instruction(nop)
                        out.append(nop)
                    inst.sync_info = mybir.SyncInfo(
                        on_wait=[waits[-1]], on_update=list(si.on_update or [])
                    )
                    changed = True
                out.append(inst)
            if changed:
                il[:] = out


def _make_tc_class(tile, mybir):
    class TC(tile.TileContext):
        def __exit__(self, *exc):
            ret = super().__exit__(*exc)
            if exc[0] is None:
                _split_multi_waits(self.nc, mybir)
            return ret

    return TC


def _mask_plan(maskb):
    """Per q-tile k-tile statuses + transposed multiplicative mask tiles.

    Returns (plan, tiles): plan[qi] = (stats, nkt) with stats[kt] in
    ('full' | 'off' | int tile index) for kt < nkt; tiles is [nt, 128, 128]
    f32 of 0/1 masks in [k, q] (transposed) layout.
    """
    tiles = []
    tile_ids = {}
    plan = []
    for qi in range(QT_TILES):
        rows = maskb[qi * 128 : (qi + 1) * 128]
        stats = []
        for kt in range(QT_TILES):
            sub = rows[:, kt * 128 : (kt + 1) * 128]
            if sub.all():
                stats.append("full")
            elif not sub.any():
                stats.append("off")
            else:
                t = np.ascontiguousarray(sub.T).astype(np.float32)
                key = t.tobytes()
                if key not in tile_ids:
                    tile_ids[key] = len(tiles)
                    tiles.append(t)
                stats.append(tile_ids[key])
        nkt = QT_TILES
        while nkt > 0 and stats[nkt - 1] == "off":
            nkt -= 1
        plan.append((tuple(stats[:nkt]), nkt))
    nt = len(tiles)
    tiles_arr = (
        np.stack(tiles).astype(np.float32) if nt else np.zeros((0, 128, 128), np.float32)
    )
    return tuple(plan), tiles_arr


def _bcast_mid(bass, ap, n):
    """[P, K] AP -> [P, n, K] AP with a step-0 middle dim (free broadcast)."""
    dims = [list(x) for x in ap.ap]
    return bass.AP(
        tensor=ap.tensor, offset=ap.offset, ap=[dims[0], [0, n]] + dims[1:]
    )


def _build(plan, nt, repeat=1, skip_cc=False, skip_p1=False):
    import concourse.bass as bass
    import concourse.mybir as mybir
    import concourse.tile as tile
    from concourse.bass import ts
    from concourse.masks import make_identity

    f32 = mybir.dt.float32
    bf16 = mybir.dt.bfloat16
    Exp = mybir.ActivationFunctionType.Exp
    TC = _make_tc_class(tile, mybir)

    nc = bass.Bass(num_devices=N_CORES)
    xT_d = nc.dram_tensor("xT", [8, 128, SHARD], bf16, kind="ExternalInput")
    cflat_d = nc.dram_tensor("cflat", [8, 128, RANK * NCMP], bf16, kind="ExternalInput")
    routersT_d = nc.dram_tensor("routersT", [8, 128, 48], bf16, kind="ExternalInput")
    wqkvT_d = nc.dram_tensor("wqkvT", [3, 128, HPC * DH], bf16, kind="ExternalInput")
    dmask_d = (
        nc.dram_tensor("dmask", [nt, 128, 128], bf16, kind="ExternalInput")
        if nt
        else None
    )
    avu_d = nc.dram_tensor("avu", [QT_TILES, 2, 128, 128], bf16, kind="ExternalOutput")
    zo_d = nc.dram_tensor("zo", [QT_TILES, 2, 2, 128], f32, kind="ExternalOutput")

    groups = [[0, 1, 2, 3], [4, 5, 6, 7]]
    CCW = 3 * TPC * 128  # columns per cc chunk: 3 routers x TPC tiles x 128

    with TC(nc) as tc:
      for _rep in range(repeat):
        with (
            tc.tile_pool(name="sb", bufs=1) as sbp,
            tc.tile_pool(name="wk2", bufs=2) as wk2,
            tc.tile_pool(name="wk4", bufs=4) as wk4,
            tc.tile_pool(name="wk8", bufs=8) as wk8,
            tc.tile_pool(name="dram", bufs=1, space="DRAM") as dramp,
        ):
            ones_sb = sbp.tile([128, 64], bf16)
            nc.vector.memset(ones_sb[:], 1.0)
            cbias = sbp.tile([128, 1], f32)
            nc.vector.memset(cbias[:], -20.0)
            # cc_in[1] is padded by 16 columns: a tiny "gate" write lands there
            # after chunk-0's gather, forcing the scheduler to order chunk-0's
            # gather DMAs ahead of the second collective on the DMA lanes
            # (otherwise their lane clocks include CC_B and they stall on it).
            cc_in = [
                dramp.tile([128, CCW + 16 * ch], bf16, name=f"cc_in{ch}")
                for ch in range(NCHUNK)
            ]
            cc_out = [
                dramp.tile([4, 128, CCW + 16 * ch], bf16, name=f"cc_out{ch}")
                for ch in range(NCHUNK)
            ]
            gate_sb = sbp.tile([128, 16], bf16)

            # attention-side constant loads (no deps -- schedule early)
            wq = sbp.tile([128, 3, HPC * DH], bf16)
            for r in range(3):
                nc.sync.dma_start(wq[:, r, :], wqkvT_d[r])
            if nt:
                masks = sbp.tile([128, nt, 128], bf16)
                for t in range(nt):
                    nc.sync.dma_start(masks[:, t, :], dmask_d[t])

            hxt = sbp.tile([128, 3, S], bf16)

            def _gather_hxt(cch):
                # gather chunk's h^T into hxt at global tile positions via
                # XBAR transpose-on-read (cc_out holds h [tok, rank]):
                # global tile of (rank g, local tile TPC*cch+lt) = 4*(TPC*cch+lt)+g
                for g4 in range(4):
                    for r in range(3):
                        for lt in range(TPC):
                            kt = 4 * (TPC * cch + lt) + g4
                            nc.sync.dma_start_transpose(
                                hxt[:, r, ts(kt, 128)],
                                cc_out[cch][g4][
                                    :, (r * TPC + lt) * 128 : (r * TPC + lt + 1) * 128
                                ],
                            )

            with (
                tc.tile_pool(name="p1ps", bufs=2, space="PSUM") as p1ps,
                tc.tile_pool(name="p1pw", bufs=1, space="PSUM") as p1pw,
            ):
                # ------------- phase 1: routing + compress -------------
                if skip_p1:
                    for ch in range(NCHUNK):
                        if not skip_cc:
                            nc.gpsimd.collective_compute(
                                "AllGather", mybir.AluOpType.bypass,
                                replica_groups=groups,
                                ins=[cc_in[ch][:]], outs=[cc_out[ch][:]],
                            )
                        else:
                            for g in range(4):
                                nc.sync.dma_start(cc_out[ch][g], cc_in[ch][:])
                        _gather_hxt(ch)
                else:
                    cfl = sbp.tile([128, 8, RANK * NCMP], bf16)
                    xt = sbp.tile([128, 8, SHARD], bf16)
                    rt = sbp.tile([128, 8, 48], bf16)
                    for dk in range(8):
                        nc.sync.dma_start(xt[:, dk, :], xT_d[dk])
                        nc.sync.dma_start(rt[:, dk, :], routersT_d[dk])
                    # ch-major cfl chunks: y matmuls for ch start after 8 small
                    # loads instead of the full 4.2MB
                    for ch in range(4):
                        for dk in range(8):
                            nc.sync.dma_start(
                                cfl[:, dk, ts(ch, 512)], cflat_d[dk][:, ts(ch, 512)]
                            )

                    ys = [
                        sbp.tile([128, RANK * NCMP], bf16, name=f"y{t}")
                        for t in range(NT4)
                    ]
                    e3s, rz3s = [], []
                    for t in range(NT4):
                        w3ps = p1pw.tile([128, 48], f32, tag="w3ps")
                        for dk in range(8):
                            nc.tensor.matmul(
                                w3ps[:], xt[:, dk, ts(t, 128)], rt[:, dk, :],
                                start=(dk == 0), stop=(dk == 7),
                            )
                        # raw exp weights (bf16) + per-token 1/z folded into h
                        e3 = wk4.tile([128, 48], bf16, tag="e3", name="e3")
                        rz3 = wk4.tile([128, 3], f32, tag="rz3", name="rz3")
                        for r in range(3):
                            z3 = wk4.tile([128, 1], f32, tag="z3")
                            nc.scalar.activation(
                                e3[:, ts(r, 16)], w3ps[:, ts(r, 16)], Exp,
                                bias=0.0, scale=1.0, accum_out=z3[:],
                            )
                            nc.vector.reciprocal(rz3[:, r : r + 1], z3[:])
                        e3s.append(e3)
                        rz3s.append(rz3)
                    for t in range(NT4):
                        e3, rz3 = e3s[t], rz3s[t]
                        cch, lt = divmod(t, TPC)
                        hs3 = [
                            wk4.tile([128, RANK], bf16, tag=f"h{r}", name=f"h{r}")
                            for r in range(3)
                        ]
                        # y[s, (r, n)] = x @ C  (r outer, n inner); each ch
                        # chunk covers ranks [32ch, 32ch+32). Combine each
                        # chunk on DVE right away so h completes ~right after
                        # the tile's last y matmul (collective starts early).
                        for ch in range(4):
                            yps = p1ps.tile([128, 512], f32, tag="yps")
                            for dk in range(8):
                                nc.tensor.matmul(
                                    yps[:], xt[:, dk, ts(t, 128)],
                                    cfl[:, dk, ts(ch, 512)],
                                    start=(dk == 0), stop=(dk == 7),
                                )
                            nc.scalar.copy(ys[t][:, ts(ch, 512)], yps[:])
                            yv = ys[t][:, ts(ch, 512)].rearrange(
                                "p (r n) -> p r n", n=NCMP
                            )
                            for r in range(3):
                                tmp = wk2.tile([128, 512], bf16, tag="tmp")
                                tv = tmp[:].rearrange("p (r n) -> p r n", n=NCMP)
                                wb = _bcast_mid(bass, e3[:, ts(r, 16)], 32)
                                nc.vector.tensor_mul(tv, yv, wb)
                                # n-sum as a binary add tree
                                for lvl in (8, 4, 2):
                                    nc.vector.tensor_add(
                                        tv[:, :, 0:lvl], tv[:, :, 0:lvl],
                                        tv[:, :, lvl : 2 * lvl],
                                    )
                                hv = hs3[r][:, ts(ch, 32)].rearrange(
                                    "p (r o) -> p r o", o=1
                                )
                                nc.vector.tensor_add(
                                    hv, tv[:, :, 0:1], tv[:, :, 1:2]
                                )
                        for r in range(3):
                            h = hs3[r]
                            nc.vector.tensor_scalar_mul(h[:], h[:], rz3[:, r : r + 1])
                            # cc_in holds h untransposed ([token, rank]); the
                            # gather side transposes via the XBAR dma.
                            nc.sync.dma_start(
                                cc_in[cch][:, (r * TPC + lt) * 128 : (r * TPC + lt + 1) * 128],
                                h[:],
                            )
                        if lt == TPC - 1:
                            # chunk cch complete -> gather it now, overlapped
                            # with the remaining tiles / chunk-(cch-1) attention
                            if not skip_cc:
                                nc.gpsimd.collective_compute(
                                    "AllGather", mybir.AluOpType.bypass,
                                    replica_groups=groups,
                                    ins=[cc_in[cch][:]], outs=[cc_out[cch][:]],
                                )
                            else:
                                for g in range(4):
                                    nc.sync.dma_start(cc_out[cch][g], cc_in[cch][:])
                            # emit this chunk's hxt gather right here, BEFORE
                            # the next chunk's collective: consumers emitted
                            # after a later collective inherit a wait on it
                            _gather_hxt(cch)
                            if cch + 1 < NCHUNK:
                                # gate: next chunk's collective input includes
                                # a pad written from this chunk's output, so
                                # the scheduler orders this chunk's gather
                                # DMAs ahead of the next collective
                                nc.sync.dma_start(
                                    gate_sb[:], cc_out[cch][0][:, 0:16]
                                )
                                nc.sync.dma_start(
                                    cc_in[cch + 1][:, CCW : CCW + 16], gate_sb[:]
                                )

            # ------------- per chunk: expand + attention -------------
            QT = sbp.tile([128, 2, S], bf16)
            KT = sbp.tile([128, 2, S], bf16)
            V = sbp.tile([128, QT_TILES, HPC * DH], bf16)
            for cch in range(NCHUNK):
                with tc.tile_pool(name=f"qkps{cch}", bufs=4, space="PSUM") as qkps:
                    # expand: Q^T, K^T (komns of chunk tokens), V tiles
                    cols = KPC * 128  # 1024 columns per chunk
                    for r, dst in ((0, QT), (1, KT)):
                        eng = nc.scalar.copy if r == 0 else nc.vector.tensor_copy
                        for half in range(2):
                            c0 = cch * cols + half * (cols // 2)
                            for pair in range(2):
                                ps = qkps.tile([128, 512], f32, tag="qk")
                                nc.tensor.matmul(
                                    ps[:], wq[:, r, ts(pair, 128)],
                                    hxt[:, r, c0 : c0 + 512],
                                    start=True, stop=True,
                                )
                                eng(dst[:, pair, c0 : c0 + 512], ps[:])
                    for st in range(KPC * cch, KPC * (cch + 1)):
                        vps = qkps.tile([128, HPC * DH], f32, tag="qk")
                        nc.tensor.matmul(
                            vps[:], hxt[:, 2, ts(st, 128)], wq[:, 2, :],
                            start=True, stop=True,
                        )
                        # alternate Act/DVE so the expand copies (critical
                        # path between collective and attention) are balanced
                        if st % 2 == 0:
                            nc.scalar.copy(V[:, st, :], vps[:])
                        else:
                            nc.vector.tensor_copy(V[:, st, :], vps[:])

                # ------------- attention + export -------------
                with (
                    tc.tile_pool(name=f"scp{cch}", bufs=2, space="PSUM") as scp,
                    tc.tile_pool(name=f"avp{cch}", bufs=1, space="PSUM") as avp,
                    tc.tile_pool(name=f"opp{cch}", bufs=2, space="PSUM") as opp,
                ):
                    for qi in range(KPC * cch, KPC * (cch + 1)):
                        stats, nkt = plan[qi]
                        acts = [kt for kt in range(nkt) if stats[kt] != "off"]
                        # all 4 heads share pts/av/zp: pts [128, 4, S];
                        # head hi = (pair hi//2, half hi%2). Scores + exp
                        # remain per pair (a head's score group must own a
                        # whole 2KB PSUM zero-region).
                        pts = wk2.tile([128, 4, S], bf16, tag="pts", name="pts")
                        # av: head pair p in its own bank (start=True zeroes
                        # the full 2KB region per partition; co-residents must
                        # be on disjoint partitions)
                        av = avp.tile([128, 1024], f32, tag="av", name="av")
                        zp = opp.tile([128, 128], f32, tag="z", name="zp")
                        # z rows on distinct 32-col groups -> 4-way concurrent
                        zrows = tuple(
                            zp[32 * hi : 32 * hi + 1, 0:128] for hi in range(4)
                        )
                        avds = tuple(
                            av[
                                64 * (hi % 2) : 64 * (hi % 2) + 64,
                                512 * (hi // 2) : 512 * (hi // 2) + 128,
                            ]
                            for hi in range(4)
                        )

                        def zav_group(g, gw):
                            # z + AV; z decoupled from AV (own bank) so the
                            # normalization tail overlaps AV
                            for j in range(gw):
                                kt = g + j
                                if stats[kt] == "off":
                                    continue
                                st_f = kt == acts[0]
                                sp_f = kt == acts[-1]
                                for hi in range(4):
                                    nc.tensor.matmul(
                                        zrows[hi],
                                        ones_sb[:, 0:1],
                                        pts[:, hi, ts(kt, 128)],
                                        start=st_f, stop=sp_f,
                                        skip_group_check=True,
                                        tile_position=(0, 32 * hi),
                                    )
                            for j in range(gw):
                                kt = g + j
                                if stats[kt] == "off":
                                    continue
                                st_f = kt == acts[0]
                                sp_f = kt == acts[-1]
                                for hi in range(4):
                                    nc.tensor.matmul(
                                        avds[hi],
                                        V[:, kt, hi * DH : hi * DH + DH],
                                        pts[:, hi, ts(kt, 128)],
                                        start=st_f, stop=sp_f,
                                        skip_group_check=True,
                                    )

                        ktgroups = [
                            (g, min(GW, nkt - g)) for g in range(0, nkt, GW)
                        ]
                        for idx, (g, gw) in enumerate(ktgroups):
                            for pair in range(2):
                                sc = scp.tile(
                                    [128, 2, GW * 128], f32, tag="sc", name="sc"
                                )
                                # transposed score tiles: [k, q] = KT^T.T @ QT
                                # adjacent matmuls use partition rows 0-63 /
                                # 64-127 -> concurrent row-tiles on HW
                                for j in range(gw):
                                    kt = g + j
                                    if stats[kt] == "off":
                                        continue
                                    for half in range(2):
                                        off = half * 64
                                        nc.tensor.matmul(
                                            sc[:, half, ts(j, 128)],
                                            KT[off : off + 64, pair, ts(kt, 128)],
                                            QT[off : off + 64, pair, ts(qi, 128)],
                                            start=True, stop=True,
                                            skip_group_check=True,
                                        )
                                # one exp covers the pair's group
                                nc.scalar.activation(
                                    pts[:, 2 * pair : 2 * pair + 2,
                                        g * 128 : (g + gw) * 128],
                                    sc[:, :, 0 : gw * 128],
                                    Exp, bias=cbias[:], scale=1.0,
                                )
                            # zero masked regions (post-exp 0/1 mask multiply)
                            # on DVE: Pool runs collectives and must not block
                            for j in range(gw):
                                kt = g + j
                                if stats[kt] not in ("off", "full"):
                                    nc.vector.tensor_mul(
                                        pts[:, :, ts(kt, 128)],
                                        pts[:, :, ts(kt, 128)],
                                        _bcast_mid(bass, masks[:, stats[kt], :], 4),
                                    )
                            # software pipeline: previous group's z+AV after
                            # this group's scores, so PE never waits on exp
                            if idx > 0:
                                zav_group(*ktgroups[idx - 1])
                        zav_group(*ktgroups[-1])
                        # ---- export unnormalized av + z (host normalizes and
                        # applies wO); all copies on DVE ----
                        zsb = wk8.tile([128, 128], f32, tag="zsb", name="zsb")
                        for hi in range(4):
                            nc.vector.tensor_copy(
                                zsb[32 * hi : 32 * hi + 1, 0:128],
                                zp[32 * hi : 32 * hi + 1, 0:128],
                            )
                        avs = wk8.tile([128, 256], bf16, tag="avs", name="avs")
                        avv = av[:].rearrange("p (b c) -> p b c", c=512)
                        nc.vector.tensor_copy(
                            avs[:].rearrange("p (b c) -> p b c", c=128),
                            avv[:, :, 0:128],
                        )
                        for pair in range(2):
                            nc.sync.dma_start(
                                avu_d[qi, pair], avs[:, ts(pair, 128)]
                            )
                            for zi in range(2):
                                hi = 2 * pair + zi
                                nc.sync.dma_start(
                                    zo_d[qi, pair, zi],
                                    zsb[32 * hi : 32 * hi + 1, 0:128],
                                )
    return nc


def _make_runner(plan, nt, repeat=1, skip_cc=False, skip_p1=False):
    """Compile the graph once and return fn(in_maps) -> list of out arrays."""
    import jax
    import numpy as np
    from jax.sharding import Mesh, PartitionSpec
    from jax.experimental.shard_map import shard_map
    import concourse.bass2jax as bass2jax
    import concourse.mybir as mybir

    nc = _build(plan, nt, repeat=repeat, skip_cc=skip_cc, skip_p1=skip_p1)
    bass2jax.install_neuronx_cc_hook()

    partition_name = nc.partition_id_tensor.name if nc.partition_id_tensor else None
    in_names, out_names, out_avals = [], [], []
    for alloc in nc.m.functions[0].allocations:
        if not isinstance(alloc, mybir.MemoryLocationSet):
            continue
        name = alloc.memorylocations[0].name
        if alloc.kind == "ExternalInput":
            if name != partition_name:
                in_names.append(name)
        elif alloc.kind == "ExternalOutput":
            out_names.append(name)
            out_avals.append(
                jax.core.ShapedArray(
                    tuple(alloc.tensor_shape), mybir.dt.np(alloc.dtype)
                )
            )
    n_params = len(in_names)
    all_names = in_names + out_names
    if partition_name is not None:
        all_names = all_names + [partition_name]

    def _body(*args):
        operands = list(args)
        if partition_name is not None:
            operands.append(bass2jax.partition_id_tensor())
        outs = bass2jax._bass_exec_p.bind(
            *operands,
            out_avals=tuple(out_avals),
            in_names=tuple(all_names),
            out_names=tuple(out_names),
            lowering_input_output_aliases=(),
            sim_require_finite=True,
            sim_require_nnan=True,
            nc=nc,
        )
        return tuple(outs)

    devices = jax.devices()[:N_CORES]
    mesh = Mesh(np.asarray(devices), ("core",))
    SHARED = {"cflat", "routersT", "dmask"}
    in_specs = tuple(
        PartitionSpec() if n in SHARED else PartitionSpec("core") for n in in_names
    ) + (PartitionSpec("core"),) * len(out_names)
    sharded = jax.jit(
        shard_map(
            _body,
            mesh=mesh,
            in_specs=in_specs,
            out_specs=(PartitionSpec("core"),) * len(out_names),
            check_rep=False,
        ),
        keep_unused=True,
    )
    zeros = [
        np.zeros((N_CORES * a.shape[0], *a.shape[1:]), a.dtype) for a in out_avals
    ]

    def make_args(in_maps, device=False):
        arrs = []
        for n in in_names:
            if n in SHARED:
                arrs.append(np.asarray(in_maps[0][n]))
            else:
                arrs.append(
                    np.concatenate([np.asarray(m[n]) for m in in_maps], axis=0)
                )
        arrs += list(zeros)
        if device:
            from jax.sharding import NamedSharding

            for i, n in enumerate(in_names):
                sh = NamedSharding(
                    mesh, PartitionSpec() if n in SHARED else PartitionSpec("core")
                )
                arrs[i] = jax.device_put(arrs[i], sh)
            sh = NamedSharding(mesh, PartitionSpec("core"))
            for i in range(len(in_names), len(arrs)):
                arrs[i] = jax.device_put(arrs[i], sh)
        return arrs

    def run(in_maps):
        outs = sharded(*make_args(in_maps))
        avu = np.asarray(outs[out_names.index("avu")])
        zo = np.asarray(outs[out_names.index("zo")])
        return (
            avu.reshape(N_CORES, QT_TILES, 2, 128, 128),
            zo.reshape(N_CORES, QT_TILES, 2, 2, 128),
        )

    run.sharded = sharded
    run.make_args = make_args
    run.out_index = out_names.index("avu")
    return run


def _prepare(inputs):
    """Host-side prep: mask plan + per-core input maps."""
    x = np.asarray(inputs["x"], np.float32)
    mask = np.asarray(inputs["mask"], bool)[0, 0]
    compress = np.asarray(inputs["compress_neurons"], np.float32)
    rQ = np.asarray(inputs["router_Q"], np.float32)
    rK = np.asarray(inputs["router_K"], np.float32)
    rV = np.asarray(inputs["router_V"], np.float32)
    wQ = np.asarray(inputs["wQ"], np.float32)
    wK = np.asarray(inputs["wK"], np.float32)
    wV = np.asarray(inputs["wV"], np.float32)

    plan, mtiles = _mask_plan(mask)
    nt = len(mtiles)

    # host-side shared prep
    import ml_dtypes

    bf = ml_dtypes.bfloat16
    cflat = np.ascontiguousarray(
        compress.transpose(1, 2, 0).reshape(8, 128, RANK * NCMP)
    ).astype(bf)  # [D, R, NC] -> d-tiles
    routersT = np.ascontiguousarray(
        np.stack([rQ, rK, rV]).transpose(2, 0, 1).reshape(8, 128, 48)
    ).astype(bf)
    wqT = wQ.T * np.float32(1.0 / np.sqrt(DH))  # fold 1/sqrt(dh) into Q
    wkT, wvT = wK.T, wV.T

    in_maps = []
    for c in range(N_CORES):
        b, g = divmod(c, 4)
        hs = slice(HPC * g * DH, HPC * g * DH + HPC * DH)
        # interleaved token shard: local tile t = global tile 4t+g
        tok = np.concatenate(
            [np.arange(128 * (4 * t + g), 128 * (4 * t + g + 1)) for t in range(NT4)]
        )
        m = {
            "xT": np.ascontiguousarray(x[b, tok, :].T)
            .reshape(8, 128, SHARD)
            .astype(bf),
            "cflat": cflat,
            "routersT": routersT,
            "wqkvT": np.ascontiguousarray(
                np.stack([wqT[:, hs], wkT[:, hs], wvT[:, hs]])
            ).astype(bf),
        }
        if nt:
            m["dmask"] = mtiles.astype(bf)
        in_maps.append(m)
    return plan, nt, in_maps


def kernel(**inputs):
    plan, nt, in_maps = _prepare(inputs)
    key = (plan, nt)
    if key not in _RUNNERS:
        _RUNNERS[key] = _make_runner(plan, nt)
    avu, zo = _RUNNERS[key](in_maps)
    wO = np.asarray(inputs["wO"], np.float32)
    out = np.zeros((B, S, D), np.float32)
    for c in range(N_CORES):
        b, g = divmod(c, 4)
        hs = slice(HPC * g * DH, HPC * g * DH + HPC * DH)
        # A[s, d] = avu / z per head; out_partial = A @ wO[:, hs].T
        a = avu[c].astype(np.float32)  # [16, 2, 128d, 128q]
        z = zo[c]  # [16, 2, 2, 128q]
        zd = np.repeat(z, DH, axis=2)  # [16, 2, 128d, 128q]
        A = (a / zd).transpose(0, 3, 1, 2).reshape(S, 2 * 128)
        out[b] += A @ wO[:, hs].T
    return out
